# revision 1
# baseline (speedup 1.0000x reference)
"""Chamfer distance loss (per-cluster, bidirectional) on 8 Trainium2 cores.

Problem: points [131072, 3] in 128 equal clusters of 1024. Per cluster c:
  d[i,j] = ||a_i - b_j||^2 ; loss_c = sum_i min_j d + sum_j min_i d
Total = sum of loss_c over clusters 0..126 (the max cluster id is excluded).

Strategy (data-parallel over clusters, 16 clusters/core):
  - Host packs, per cluster, two K=5 operand matrices:
        A_op = [ax; ay; az; 1; aa]   (5 x 1024)
        B_op = [-2bx; -2by; -2bz; bb; 1]
    so that A_op^T @ B_op == d directly (PE emits the distance matrix).
  - Pass 1: lhsT = A_op chunk (i on PSUM partitions), rhs = B_op -> d[i,j].
  - Pass 2: lhsT = B_op chunk, rhs = A_op -> d^T (min over i becomes a row min).
  - Row mins via one DVE tensor_tensor_reduce per [128,1024] block:
    accum_out = min over both j-halves (op0=min folds halves, op1=min reduces).
  - Device outputs raw block mins [128 lanes, 256 cols]; host sums and masks.
"""

import numpy as np

C = 128          # clusters
P = 1024         # points per cluster
DIM = 3
K = 13           # augmented contraction dim (split-fp16 rows)
N_CORES = 8
CPC = C // N_CORES   # clusters per core (16)
ICH = P // 128       # i-chunks per cluster (8)
OUT_COLS = 2 * CPC * ICH  # 256

_cache = {}


def _build():
    import concourse.bacc as bacc
    import concourse.mybir as mybir
    from concourse.tile import TileContext

    nc = bacc.Bacc(
        "TRN2", target_bir_lowering=False, debug=False, num_devices=N_CORES)
    f32 = mybir.dt.float32
    f16 = mybir.dt.float16

    a_d = nc.dram_tensor("a_op", [K, CPC * P], f16, kind="ExternalInput")
    b_d = nc.dram_tensor("b_op", [K, CPC * P], f16, kind="ExternalInput")
    out_d = nc.dram_tensor("out", [128, OUT_COLS], f32, kind="ExternalOutput")

    with TileContext(nc) as tc:
        with (
            tc.tile_pool(name="const", bufs=1) as cpool,
            tc.tile_pool(name="psum", bufs=4, space="PSUM") as ppool,
            tc.tile_pool(name="scratch", bufs=4) as spool,
        ):
            a_t = cpool.tile([K, CPC * P], f16)
            b_t = cpool.tile([K, CPC * P], f16)
            # cluster-aligned chunks so each matmul depends on one DMA
            nq = 2
            w = CPC * P // nq
            for q in range(nq):
                nc.sync.dma_start(
                    out=a_t[:, q * w:(q + 1) * w], in_=a_d[:, q * w:(q + 1) * w])
                nc.sync.dma_start(
                    out=b_t[:, q * w:(q + 1) * w], in_=b_d[:, q * w:(q + 1) * w])
            mins = cpool.tile([128, OUT_COLS], f32)

            for dirn in range(2):
                s_t, m_t = (a_t, b_t) if dirn == 0 else (b_t, a_t)
                for c in range(CPC):
                    cs = c * P
                    for ic in range(ICH):
                        ps = ppool.tile([128, P], f32, tag="ps")
                        lhsT = s_t[:, cs + ic * 128:cs + (ic + 1) * 128]
                        nc.tensor.matmul(
                            ps[:, 0:512], lhsT, m_t[:, cs:cs + 512],
                            start=True, stop=True)
                        nc.tensor.matmul(
                            ps[:, 512:1024], lhsT, m_t[:, cs + 512:cs + P],
                            start=True, stop=True)
                        col = (dirn * CPC + c) * ICH + ic
                        # single DVE reduce over the 2-bank PSUM tile
                        # (tensor_tensor_reduce would halve DVE time but
                        # crashes TRN2)
                        nc.vector.tensor_reduce(
                            out=mins[:, col:col + 1], in_=ps[:],
                            axis=mybir.AxisListType.X, op=mybir.AluOpType.min)

            nc.sync.dma_start(out=out_d[:], in_=mins[:])
    nc.compile()
    return nc


def _split(x):
    """fp32 -> (hi, lo) fp16 pair with x ~= hi + lo."""
    hi = x.astype(np.float16)
    lo = (x - hi.astype(np.float32)).astype(np.float16)
    return hi, lo


def _prep(input_points, output_points):
    a = np.ascontiguousarray(input_points, dtype=np.float32).reshape(C, P, DIM)
    b = np.ascontiguousarray(output_points, dtype=np.float32).reshape(C, P, DIM)
    aa = np.einsum("cpd,cpd->cp", a, a).astype(np.float32)
    bb = np.einsum("cpd,cpd->cp", b, b).astype(np.float32)

    at = a.transpose(0, 2, 1)            # [C,3,P]
    bt2 = -2.0 * b.transpose(0, 2, 1)    # [C,3,P]  (B = -2b)
    ah, al = _split(at)
    bh, bl = _split(bt2)
    aah, aal = _split(aa)
    bbh, bbl = _split(bb)

    # d = sum_k A[k,i] * B[k,j]:
    #   ah.Bh + al.Bh + ah.Bl  (= -2ab)   rows 0-2, 3-5, 6-8
    #   1*bbh + 1*bbl                      rows 9, 10
    #   aah*1 + aal*1                      rows 11, 12
    a_op = np.empty((C, K, P), np.float16)
    a_op[:, 0:3] = ah
    a_op[:, 3:6] = al
    a_op[:, 6:9] = ah
    a_op[:, 9:11] = 1.0
    a_op[:, 11] = aah
    a_op[:, 12] = aal

    b_op = np.empty((C, K, P), np.float16)
    b_op[:, 0:3] = bh
    b_op[:, 3:6] = bh
    b_op[:, 6:9] = bl
    b_op[:, 9] = bbh
    b_op[:, 10] = bbl
    b_op[:, 11:13] = 1.0

    in_maps = []
    for i in range(N_CORES):
        sl = slice(i * CPC, (i + 1) * CPC)
        # [CPC, K, P] -> [K, CPC*P]
        in_maps.append({
            "a_op": np.ascontiguousarray(
                a_op[sl].transpose(1, 0, 2).reshape(K, CPC * P)),
            "b_op": np.ascontiguousarray(
                b_op[sl].transpose(1, 0, 2).reshape(K, CPC * P)),
        })
    return in_maps


def run(inputs, trace=False, trace_kwargs=None):
    """Returns (loss ndarray shape (), BassKernelResults)."""
    from concourse.bass_utils import run_bass_kernel_spmd

    if "nc" not in _cache:
        _cache["nc"] = _build()
    nc = _cache["nc"]

    in_maps = _prep(inputs["input_points"], inputs["output_points"])
    res = run_bass_kernel_spmd(
        nc, in_maps, list(range(N_CORES)),
        trace=trace, **(trace_kwargs or {}))

    # out[core]: [128, 256]; col = (dir*CPC + c)*ICH + ic; sum all lanes/chunks
    per_cluster = np.concatenate([
        res.results[i]["out"].reshape(128, 2, CPC, ICH).sum(
            axis=(0, 1, 3), dtype=np.float64)
        for i in range(N_CORES)
    ])  # [C]

    nb = int(np.max(inputs["input_clusters"]))
    mask = np.arange(C) < nb
    total = np.float32(per_cluster[mask].sum())
    return np.array(total, dtype=np.float32), res


def kernel(input_points, input_clusters, output_points, output_clusters):
    loss, _ = run({
        "input_points": input_points,
        "input_clusters": input_clusters,
        "output_points": output_points,
        "output_clusters": output_clusters,
    })
    return loss



# revision 2
# speedup vs baseline: 1.5715x; 1.5715x over previous
"""Chamfer distance loss (per-cluster, bidirectional) on 8 Trainium2 cores.

Problem: points [131072, 3] in 128 equal clusters of 1024. Per cluster c:
  d[i,j] = ||a_i - b_j||^2 ; loss_c = sum_i min_j d + sum_j min_i d
Total = sum of loss_c over clusters 0..126 (the max cluster id is excluded).

Strategy (data-parallel over clusters, 16 clusters/core), single matmul
pass per cluster computing NEGATED distances so mins become maxes:
  - Host packs per cluster two K=13 operand matrices (split-fp16 for
    accuracy); B side negated so PE emits -d directly into PSUM f32.
  - Per cluster: 8 i-chunks, 16 matmuls of [128, 512] -> 8 PSUM tiles.
  - Act (scalar) engine converts each PSUM tile into one slot of a
    batched SBUF f16 tile T8 [128, 8, 1024] (the only engine besides
    DVE that may read PSUM; frees PSUM quickly).
  - DVE dir-1 (min over j): batched j-half fold tree over all 8 chunks
    (tensor_tensor max, f16 2x mode) + one segmented reduce ->
    rowmax [128, 8] per cluster.
  - DVE dir-2 (min over i): fold tree across the 8 chunk sub-tiles ->
    running tile r [128, 1024] f16.
  - GpSimd partition_all_reduce(max) on r -> colmax broadcast; DMA out
    lane 0. (Needs the 'attn' ucode library.)
  - Host: loss = -sum(masked rowmax + colmax sums).

Notes from HW probing (TRN2):
  - tensor_tensor_reduce faults the device at runtime -> unusable.
  - Two-input ops may read at most ONE operand from PSUM; GpSimd may
    not touch PSUM at all; GpSimd has no TensorTensor opcode.
  - DVE tensor_tensor on packed f16 runs in 2x_1p mode (0.5 cyc/elem);
    tensor_reduce has no fast mode -> fold trees + small final reduce.
"""

import numpy as np

C = 128          # clusters
P = 1024         # points per cluster
DIM = 3
K = 13           # augmented contraction dim (split-fp16 rows)
N_CORES = 8
CPC = C // N_CORES   # clusters per core (16)
ICH = P // 128       # i-chunks per cluster (8)

_cache = {}


def _build():
    import concourse.bacc as bacc
    import concourse.mybir as mybir
    import concourse.bass_isa as bass_isa
    from concourse import library_config
    from concourse.tile import TileContext

    nc = bacc.Bacc(
        "TRN2", target_bir_lowering=False, debug=False, num_devices=N_CORES)
    f32 = mybir.dt.float32
    f16 = mybir.dt.float16
    mx = mybir.AluOpType.max

    a_d = nc.dram_tensor("a_op", [K, CPC * P], f16, kind="ExternalInput")
    b_d = nc.dram_tensor("b_op", [K, CPC * P], f16, kind="ExternalInput")
    rmax_d = nc.dram_tensor(
        "rowmax", [128, CPC * ICH], f32, kind="ExternalOutput")
    cmax_d = nc.dram_tensor("colmax", [CPC, P], f32, kind="ExternalOutput")

    with TileContext(nc) as tc:
        with (
            tc.tile_pool(name="const", bufs=1) as cpool,
            tc.tile_pool(name="psum", bufs=3, space="PSUM") as ppool,
            tc.tile_pool(name="tbat", bufs=2) as tpool,
            tc.tile_pool(name="tree", bufs=2) as ypool,
        ):
            a_t = cpool.tile([K, CPC * P], f16)
            b_t = cpool.tile([K, CPC * P], f16)
            nq = 2
            w = CPC * P // nq
            for q in range(nq):
                nc.sync.dma_start(
                    out=a_t[:, q * w:(q + 1) * w], in_=a_d[:, q * w:(q + 1) * w])
                nc.sync.dma_start(
                    out=b_t[:, q * w:(q + 1) * w], in_=b_d[:, q * w:(q + 1) * w])
            rowmax = cpool.tile([128, CPC * ICH], f32)

            nc.gpsimd.load_library(library_config.attn)

            for c in range(CPC):
                cs = c * P
                t8 = tpool.tile([128, ICH, P], f16, tag="t8")
                for ic in range(ICH):
                    ps = ppool.tile([128, P], f32, tag="ps")
                    lhsT = a_t[:, cs + ic * 128:cs + (ic + 1) * 128]
                    nc.tensor.matmul(
                        ps[:, 0:512], lhsT, b_t[:, cs:cs + 512],
                        start=True, stop=True)
                    nc.tensor.matmul(
                        ps[:, 512:1024], lhsT, b_t[:, cs + 512:cs + P],
                        start=True, stop=True)
                    nc.scalar.copy(out=t8[:, ic, :], in_=ps[:])

                # dir-1: rowmax per i: batched j-half fold tree + seg reduce
                y1 = ypool.tile([128, ICH, 512], f16, tag="y1")
                nc.vector.tensor_tensor(
                    out=y1[:], in0=t8[:, :, 0:512], in1=t8[:, :, 512:1024],
                    op=mx)
                y2 = ypool.tile([128, ICH, 256], f16, tag="y2")
                nc.vector.tensor_tensor(
                    out=y2[:], in0=y1[:, :, 0:256], in1=y1[:, :, 256:512],
                    op=mx)
                y3 = ypool.tile([128, ICH, 128], f16, tag="y3")
                nc.vector.tensor_tensor(
                    out=y3[:], in0=y2[:, :, 0:128], in1=y2[:, :, 128:256],
                    op=mx)
                y4 = ypool.tile([128, ICH, 64], f16, tag="y4")
                nc.vector.tensor_tensor(
                    out=y4[:], in0=y3[:, :, 0:64], in1=y3[:, :, 64:128],
                    op=mx)
                nc.vector.tensor_reduce(
                    out=rowmax[:, c * ICH:(c + 1) * ICH], in_=y4[:],
                    axis=mybir.AxisListType.X, op=mx)

                # dir-2: colmax per j: fold across the 8 chunk sub-tiles
                w4 = ypool.tile([128, 4, P], f16, tag="w4")
                nc.vector.tensor_tensor(
                    out=w4[:], in0=t8[:, 0:4, :], in1=t8[:, 4:8, :], op=mx)
                w2 = ypool.tile([128, 2, P], f16, tag="w2")
                nc.vector.tensor_tensor(
                    out=w2[:], in0=w4[:, 0:2, :], in1=w4[:, 2:4, :], op=mx)
                r = ypool.tile([128, P], f16, tag="r")
                nc.vector.tensor_tensor(
                    out=r[:], in0=w2[:, 0, :], in1=w2[:, 1, :], op=mx)

                parout = ypool.tile([128, P], f32, tag="parout")
                nc.gpsimd.partition_all_reduce(
                    out_ap=parout[:], in_ap=r[:], channels=128,
                    reduce_op=bass_isa.ReduceOp.max)
                nc.sync.dma_start(out=cmax_d[c:c + 1, :], in_=parout[0:1, :])

            nc.sync.dma_start(out=rmax_d[:], in_=rowmax[:])
    nc.compile()
    return nc


def _split(x):
    """fp32 -> (hi, lo) fp16 pair with x ~= hi + lo."""
    hi = x.astype(np.float16)
    lo = (x - hi.astype(np.float32)).astype(np.float16)
    return hi, lo


def _prep(input_points, output_points):
    a = np.ascontiguousarray(input_points, dtype=np.float32).reshape(C, P, DIM)
    b = np.ascontiguousarray(output_points, dtype=np.float32).reshape(C, P, DIM)
    aa = np.einsum("cpd,cpd->cp", a, a).astype(np.float32)
    bb = np.einsum("cpd,cpd->cp", b, b).astype(np.float32)

    at = a.transpose(0, 2, 1)            # [C,3,P]
    bt2 = -2.0 * b.transpose(0, 2, 1)    # [C,3,P]  (B = -2b)
    ah, al = _split(at)
    bh, bl = _split(bt2)
    aah, aal = _split(aa)
    bbh, bbl = _split(bb)

    # -d = sum_k A[k,i] * (-B_orig[k,j]): negate the whole B side.
    a_op = np.empty((C, K, P), np.float16)
    a_op[:, 0:3] = ah
    a_op[:, 3:6] = al
    a_op[:, 6:9] = ah
    a_op[:, 9:11] = 1.0
    a_op[:, 11] = aah
    a_op[:, 12] = aal

    b_op = np.empty((C, K, P), np.float16)
    b_op[:, 0:3] = -bh
    b_op[:, 3:6] = -bh
    b_op[:, 6:9] = -bl
    b_op[:, 9] = -bbh
    b_op[:, 10] = -bbl
    b_op[:, 11:13] = -1.0

    in_maps = []
    for i in range(N_CORES):
        sl = slice(i * CPC, (i + 1) * CPC)
        in_maps.append({
            "a_op": np.ascontiguousarray(
                a_op[sl].transpose(1, 0, 2).reshape(K, CPC * P)),
            "b_op": np.ascontiguousarray(
                b_op[sl].transpose(1, 0, 2).reshape(K, CPC * P)),
        })
    return in_maps


def run(inputs, trace=False, trace_kwargs=None):
    """Returns (loss ndarray shape (), BassKernelResults)."""
    from concourse.bass_utils import run_bass_kernel_spmd

    if "nc" not in _cache:
        _cache["nc"] = _build()
    nc = _cache["nc"]

    in_maps = _prep(inputs["input_points"], inputs["output_points"])
    res = run_bass_kernel_spmd(
        nc, in_maps, list(range(N_CORES)),
        trace=trace, **(trace_kwargs or {}))

    # rowmax[core]: [128, CPC*ICH]; col c*8+t = max_j(-d) for i-chunk t
    # colmax[core]: [CPC, 1024]; row c = max_i(-d) per j
    per_cluster = np.empty(C, np.float64)
    for i in range(N_CORES):
        rm = res.results[i]["rowmax"].astype(np.float64)
        cm = res.results[i]["colmax"].astype(np.float64)
        l1 = rm.reshape(128, CPC, ICH).sum(axis=(0, 2))  # [CPC]
        l2 = cm.sum(axis=1)                              # [CPC]
        per_cluster[i * CPC:(i + 1) * CPC] = -(l1 + l2)

    nb = int(np.max(inputs["input_clusters"]))
    mask = np.arange(C) < nb
    total = np.float32(per_cluster[mask].sum())
    return np.array(total, dtype=np.float32), res


def kernel(input_points, input_clusters, output_points, output_clusters):
    loss, _ = run({
        "input_points": input_points,
        "input_clusters": input_clusters,
        "output_points": output_points,
        "output_clusters": output_clusters,
    })
    return loss
